# revision 12
# baseline (speedup 1.0000x reference)
"""Multi-head self-attention Trainium2 kernel (8 NeuronCores, SPMD).

Problem: B=2, N=4096, D=512, H=8 heads of dim 64.
  qkv = x @ qkv_w.T + qkv_b ; per-head attention with softmax(QK^T/8) ;
  out = attn @ out_w.T + out_b

Sharding: 16 (batch, head) pairs -> 8 cores, each core owns one batch b and
one head-PAIR (2 adjacent heads = a 128-row slice of the qkv projections).
Each core computes the full attention for its 2 heads over all 4096 rows and
a partial output projection; the host sums the 4 per-batch partials and adds
the (folded) biases.

Pipeline design (v2 — dual-engine softmax + fp8 PV):
  The kernel is exp-throughput-bound: 33.5M exps/core. ScalarE ACT does
  head0's scores, the Vector engine does head1's via a custom 7-stage DVE op
  computing ((y+u)^2+v)^4*(y+a)^4 = K*exp(S) (monic cubic in y = alpha*S,
  alpha folded into head1's Q projection; the per-head softmax normalization
  cancels K). Both engines write p in fp8-e4m3; V is also e4m3, packed
  [128,16,2,80] so the PV matmul runs DoubleRow (2 key-tiles, 256 virtual
  contraction rows, 0.5 cy/row) — PE cost of attention halves.
  Scores S stay bf16xbf16->fp32; the two heads' S matmuls are row-packed at
  tile_position 0/64 and run concurrently. Softmax denominators ride a fused
  ones-column in V; normalization uses a bf16 PE outer-product broadcast,
  DVE reciprocal_approx_fast, and a gpsimd multiply (the only engine with
  slack). Finalize work (PSUM evictions, y-projection evictions, bias adds)
  is placed on ScalarE, which has more slack than DVE in steady state.
"""

import os
import numpy as np
import ml_dtypes

B, N, D, H, HD = 2, 4096, 512, 8, 64
NCORES = 8
KT_TILES = 4      # D / 128 contraction tiles
JT = 32           # N / 128 key tiles
JP = 16           # N / 256 key-tile pairs
ICH = 8           # N / 512 query chunks
P = 128

MODE = "v2-fp8pv"

# exp cubic constants: q(y)=(y+CA)((y+CU)^2+CV); q^4 = K*exp(y/ALPHA),
# max rel err 6.5e-3 over |S|<=3.  K=3.1575 cancels in softmax.
ALPHA = 0.15
CA = 1.02637385
CU = 0.45424912
CV = 1.09069782

_BUILD_CACHE = {}


def _register_exp_op():
    import concourse.dve_ops as dve_ops
    for op in dve_ops.OPS:
        if op.name == "EXP4_POLY_ANT":
            return op
    from concourse.dve_spec import Spec, Src0, C0, C1, C2, sq, lower, _has_src1
    from concourse.dve_uop import DveOpSpec

    t = Src0 + C0
    body = sq(sq((sq(t) + C1) * (Src0 + C2)))

    def ref(in0, in1, s0, s1, imm2):
        x = in0.astype(np.float32)
        return ((((x + s0) ** 2 + s1) * (x + imm2)) ** 4).astype(np.float32)

    spec = Spec(body=body, reference=ref)
    name = "EXP4_POLY_ANT"
    row = max(dve_ops._SUB_OPCODE_FOR_NAME.values()) + 1
    assert row < 0x20
    dve_ops._SUB_OPCODE_FOR_NAME[name] = row
    shas = {}
    for ver in ("v3", "v4"):
        try:
            uops = lower(spec, ver=ver)
            tmp = DveOpSpec(name=name, opcode=row, uops=uops,
                            rd1_en=_has_src1(spec))
            shas[ver] = tmp.sha(ver)
        except Exception:
            pass
    op = dve_ops.DveOp(name, spec, subdim=False, uops_sha=shas)
    dve_ops.OPS.append(op)
    dve_ops.CUSTOM_DVE_SPECS[name] = spec
    return op


def _build():
    """Build (and cache) the compiled Bass program for all cores (SPMD)."""
    if "k" in _BUILD_CACHE:
        return _BUILD_CACHE["k"]

    import concourse.bacc as bacc
    import concourse.mybir as mybir
    import concourse.tile as tile
    from concourse.bass import _add_dep_helper
    from contextlib import ExitStack

    f32 = mybir.dt.float32
    bf16 = mybir.dt.bfloat16
    e4 = mybir.dt.float8e4
    Exp = mybir.ActivationFunctionType.Exp
    Copy = mybir.ActivationFunctionType.Copy
    Ident = mybir.ActivationFunctionType.Identity
    DR = mybir.MatmulPerfMode.DoubleRow
    EXP4 = _register_exp_op()

    nc = bacc.Bacc(None, target_bir_lowering=False)
    xt_d = nc.dram_tensor("xt", [KT_TILES, P, N], bf16, kind="ExternalInput")
    wqt_d = nc.dram_tensor("wqt", [KT_TILES, P, P], bf16, kind="ExternalInput")
    wkt_d = nc.dram_tensor("wkt", [KT_TILES, P, P], bf16, kind="ExternalInput")
    wvt_d = nc.dram_tensor("wvt", [KT_TILES, P, P], bf16, kind="ExternalInput")
    wot_d = nc.dram_tensor("wot", [2, HD, D], bf16, kind="ExternalInput")
    bq_d = nc.dram_tensor("bq", [P, 1], f32, kind="ExternalInput")
    bk_d = nc.dram_tensor("bk", [P, 1], f32, kind="ExternalInput")
    yp_d = nc.dram_tensor("yp", [KT_TILES, P, N], bf16, kind="ExternalOutput")

    def ics(i):
        return slice(i * 512, (i + 1) * 512)

    def jts(j):
        return slice(j * P, (j + 1) * P)

    def mts(m):
        return slice(m * P, (m + 1) * P)

    with tile.TileContext(nc) as tc, ExitStack() as ctx:
        const = ctx.enter_context(tc.tile_pool(name="const", bufs=1))
        # PSUM budget (8 banks): sp 2x[128,1024]f32 = 4, o-pool 2x[65,512] = 2,
        # fp scratch 2x(<=1 bank) = 2.
        sp0 = ctx.enter_context(tc.tile_pool(name="sp0", bufs=2, space="PSUM"))
        sp1 = ctx.enter_context(tc.tile_pool(name="sp1", bufs=2, space="PSUM"))
        op = ctx.enter_context(tc.tile_pool(name="opool", bufs=2, space="PSUM"))
        fp = ctx.enter_context(tc.tile_pool(name="fpool", bufs=2, space="PSUM"))
        pp0 = ctx.enter_context(tc.tile_pool(name="pp0", bufs=3))
        pp1 = ctx.enter_context(tc.tile_pool(name="pp1", bufs=3))
        yep0 = ctx.enter_context(tc.tile_pool(name="yep0", bufs=2))
        yep1 = ctx.enter_context(tc.tile_pool(name="yep1", bufs=2))
        osp0 = ctx.enter_context(tc.tile_pool(name="osp0", bufs=2))
        osp1 = ctx.enter_context(tc.tile_pool(name="osp1", bufs=2))
        rbp = ctx.enter_context(tc.tile_pool(name="rbpool", bufs=2))

        xt = const.tile([P, KT_TILES, N], bf16, tag="xt")
        wqt = const.tile([P, KT_TILES, P], bf16, tag="wqt")
        wkt = const.tile([P, KT_TILES, P], bf16, tag="wkt")
        wvt = const.tile([P, KT_TILES, P], bf16, tag="wvt")
        for k in range(KT_TILES):
            nc.gpsimd.dma_start(wqt[:, k, :], wqt_d[k])
            nc.gpsimd.dma_start(wkt[:, k, :], wkt_d[k])
            nc.scalar.dma_start(wvt[:, k, :], wvt_d[k])
        # x^T in column-major chunk order on ONE queue: the first column
        # blocks (all k-tiles) land at ~12% of the transfer so Q/K projections
        # and the attention loop start early.
        XCH = 1024
        for c in range(N // XCH):
            for k in range(KT_TILES):
                nc.sync.dma_start(xt[:, k, c * XCH:(c + 1) * XCH],
                                  xt_d[k][:, c * XCH:(c + 1) * XCH])
        wot = const.tile([HD, 2, D], bf16, tag="wot")
        for h in range(2):
            nc.scalar.dma_start(wot[:, h, :], wot_d[h])
        bq = const.tile([P, 1], f32, tag="bq")
        bk = const.tile([P, 1], f32, tag="bk")
        nc.gpsimd.dma_start(bq[:], bq_d[:])
        nc.gpsimd.dma_start(bk[:], bk_d[:])

        QT = const.tile([P, N], bf16, tag="QT")
        KT = const.tile([P, N], bf16, tag="KT")
        # V in fp8, DoubleRow pair layout: [p, key-pair, ko, 64 v + 1 ones + pad]
        Vp0 = const.tile([P, JP, 2, 80], e4, tag="Vp0")
        Vp1 = const.tile([P, JP, 2, 80], e4, tag="Vp1")
        OT0 = const.tile([HD, N], bf16, tag="OT0")
        OT1 = const.tile([HD, N], bf16, tag="OT1")
        onesb = const.tile([65, HD], bf16, tag="onesb")
        nc.vector.memset(onesb[64:65, :], 1.0)
        actwarm = const.tile([1, 1], f32, tag="actwarm")
        nc.vector.memset(actwarm[:], 0.0)
        nc.scalar.activation(actwarm[:], actwarm[:], Exp)
        for _ko in range(2):
            nc.vector.memset(Vp0[:, :, _ko, 64:65], 1.0)
            nc.vector.memset(Vp1[:, :, _ko, 64:65], 1.0)

        # ---- projection units ----
        def qproj_unit(ic):
            ps = fp.tile([P, 512], f32, tag="fp", name=f"qp_{ic}")
            for k in range(KT_TILES):
                nc.tensor.matmul(ps[:], wqt[:, k, :], xt[:, k, ics(ic)],
                                 start=(k == 0), stop=(k == KT_TILES - 1))
            nc.vector.tensor_scalar_add(QT[:, ics(ic)], ps[:], bq[:, 0:1])

        def qproj_quarter(ic, q, after=None):
            # one [128,128] N-slice of the Q^T projection; bias-add on ScalarE
            qs = slice(ics(ic).start + 128 * q, ics(ic).start + 128 * (q + 1))
            ps = fp.tile([P, 128], f32, tag="fp", name=f"qq_{ic}_{q}")
            for k in range(KT_TILES):
                mm = nc.tensor.matmul(ps[:], wqt[:, k, :], xt[:, k, qs],
                                      start=(k == 0), stop=(k == KT_TILES - 1))
                if after is not None and k == 0:
                    _add_dep_helper(mm.ins, after.ins, sync=False,
                                    reason="defer qproj behind attention")
            if q < 2:
                nc.scalar.activation(QT[:, qs], ps[:], Ident, bias=bq[:, 0:1])
            else:
                nc.vector.tensor_scalar_add(QT[:, qs], ps[:], bq[:, 0:1])

        def kproj_unit(jc, eng="v"):
            ps = fp.tile([P, 512], f32, tag="fp", name=f"kp_{jc}")
            for k in range(KT_TILES):
                nc.tensor.matmul(ps[:], wkt[:, k, :], xt[:, k, ics(jc)],
                                 start=(k == 0), stop=(k == KT_TILES - 1))
            if eng == "v":
                nc.vector.tensor_scalar_add(KT[:, ics(jc)], ps[:], bk[:, 0:1])
            else:
                nc.scalar.activation(KT[:, ics(jc)], ps[:], Ident, bias=bk[:, 0:1])

        def vproj_unit(jt):
            # V[jt] natural layout -> fp8 pair-packed slabs (one evict per
            # engine so neither stalls the exp cadence)
            jp_, ko = divmod(jt, 2)
            ps = fp.tile([P, P], f32, tag="fp", name=f"vp_{jt}")
            for k in range(KT_TILES):
                nc.tensor.matmul(ps[:], xt[:, k, jts(jt)], wvt[:, k, :],
                                 start=(k == 0), stop=(k == KT_TILES - 1))
            nc.scalar.activation(Vp0[:, jp_, ko, 0:64], ps[:, 0:64], Copy)
            nc.vector.tensor_copy(Vp1[:, jp_, ko, 0:64], ps[:, 64:128])

        # ---- finalize quanta ----
        def finalize_a(ic, o0, o1):
            # evict unnormalized O^T (+denominator row 64) to bf16 SBUF.
            # ScalarE for head0, DVE for head1 (one each, keeps cadence).
            os0 = osp0.tile([65, 512], bf16, tag="os0", name=f"os0_{ic}")
            os1 = osp1.tile([65, 512], bf16, tag="os1", name=f"os1_{ic}")
            nc.scalar.activation(os0[:], o0[:], Copy)
            nc.vector.tensor_copy(os1[:], o1[:])
            return (os0, os1)

        def norm_quantum(ic, st, h):
            # one head: broadcast denominator via bf16 PE outer product,
            # approx-reciprocal on DVE, normalize-multiply on gpsimd.
            os_ = st[h]
            OTt = (OT0, OT1)[h]
            rb = fp.tile([HD, 512], f32, tag="fp", name=f"rb_{ic}_{h}")
            nc.tensor.matmul(rb[:], onesb[64:65, :], os_[64:65, :],
                             start=True, stop=True, tile_position=(64, 0))
            rbs = rbp.tile([HD, 512], f32, tag="rbs", name=f"rbs_{ic}_{h}")
            nc.vector.reciprocal_approx_fast(out=rbs[:], in_=rb[:])
            nc.gpsimd.tensor_mul(OTt[:, ics(ic)], os_[0:64, :], rbs[:])

        def yproj_mm(ic, mt, after=None):
            # one [128,512] slice of the partial output projection (PE only)
            hs = ics(ic)
            yps = fp.tile([P, 512], f32, tag="fp", name=f"yp_{ic}_{mt}")
            mm = nc.tensor.matmul(yps[:], wot[:, 0, mts(mt)], OT0[:, hs],
                                  start=True, stop=False)
            if after is not None:
                _add_dep_helper(mm.ins, after.ins, sync=False,
                                reason="defer finalize yproj behind attention")
            nc.tensor.matmul(yps[:], wot[:, 1, mts(mt)], OT1[:, hs],
                             start=False, stop=True)
            return yps

        def yproj_evict(ic, mt, yps, eng):
            if eng == "s":
                ye = yep0.tile([P, 512], bf16, tag="ye0", name=f"ye_{ic}_{mt}")
                nc.scalar.activation(ye[:], yps[:], Copy)
            else:
                ye = yep1.tile([P, 512], bf16, tag="ye1", name=f"ye_{ic}_{mt}")
                nc.vector.tensor_copy(ye[:], yps[:])
            nc.sync.dma_start(yp_d[mt, :, ics(ic)], ye[:])

        # prologue: K chunk 0 first (needed by the first S tiles), then Q
        kproj_unit(0)
        qproj_unit(0)
        kproj_unit(1)

        # finalize schedule within the NEXT chunk (jt slots)
        NORM_JT = {2: 0, 3: 1}
        YP_MM_JT = {10: 0, 12: 1, 14: 2, 16: 3}
        YP_EV_JT = {12: 0, 14: 1, 16: 2, 18: 3}

        otiles = {}
        pend_pv = None       # (p3, jp, ic) pair whose PV is not yet emitted
        pend_b = None
        for ic in range(ICH):
            otiles[ic] = (op.tile([65, 512], f32, tag="o", name=f"o0_{ic}"),
                          op.tile([65, 512], f32, tag="o", name=f"o1_{ic}"))
            cur_p3 = None
            for jt in range(JT):
                jp_, ko = divmod(jt, 2)
                s0 = sp0.tile([P, 512], f32, tag="s0")
                s1 = sp1.tile([P, 512], f32, tag="s1")
                last_s = nc.tensor.matmul(s0[:], KT[0:64, jts(jt)],
                                          QT[0:64, ics(ic)],
                                          start=True, stop=True,
                                          tile_position=(0, 0))
                nc.tensor.matmul(s1[:], KT[64:128, jts(jt)],
                                 QT[64:128, ics(ic)],
                                 start=True, stop=True, tile_position=(64, 0))
                if ko == 0:
                    cp0 = pp0.tile([P, 2, 512], e4, tag="p0")
                    cp1 = pp1.tile([P, 2, 512], e4, tag="p1")
                nc.scalar.activation(cp0[:, ko, :], s0[:], Exp)
                nc.vector._custom_dve(EXP4, out=cp1[:, ko, :], in0=s1[:],
                                      s0=CU, s1=CV, imm2=CA)
                if ko == 1:
                    if pend_pv is not None:
                        pq0, pq1, pjp, pic = pend_pv
                        o0, o1 = otiles[pic]
                        nc.tensor.matmul(o0[:], Vp0[:, pjp, :, 0:65],
                                         pq0[:, :, :], perf_mode=DR,
                                         start=(pjp == 0), stop=(pjp == JP - 1))
                        nc.tensor.matmul(o1[:], Vp1[:, pjp, :, 0:65],
                                         pq1[:, :, :], perf_mode=DR,
                                         start=(pjp == 0), stop=(pjp == JP - 1))
                        if pjp == JP - 1:
                            pend_b = (pic, finalize_a(pic, o0, o1), [None] * 4)
                    pend_pv = (cp0, cp1, jp_, ic)
                # deferred projection work, spread across the loop
                if ic == 0:
                    if jt == 0:
                        vproj_unit(0)
                        vproj_unit(1)
                    elif jt <= JT - 2:
                        vproj_unit(jt + 1)
                    if jt < 24 and jt % 4 == 0:
                        kproj_unit(2 + jt // 4, eng="s" if jt % 8 else "v")
                if pend_b is not None:
                    bic, st, ypss = pend_b
                    h = NORM_JT.get(jt)
                    if h is not None:
                        norm_quantum(bic, st, h)
                    mt = YP_MM_JT.get(jt)
                    if mt is not None:
                        ypss[mt] = yproj_mm(bic, mt, after=last_s)
                    mt = YP_EV_JT.get(jt)
                    if mt is not None:
                        yproj_evict(bic, mt, ypss[mt], "s" if mt % 2 == 0 else "v")
                        if mt == 3:
                            pend_b = None
                if 20 <= jt < 24 and ic + 1 < ICH:
                    qproj_quarter(ic + 1, jt - 20, after=last_s)
        # drain the pipeline tail
        pq0, pq1, pjp, pic = pend_pv
        o0, o1 = otiles[pic]
        nc.tensor.matmul(o0[:], Vp0[:, pjp, :, 0:65], pq0[:, :, :],
                         perf_mode=DR, start=(pjp == 0), stop=(pjp == JP - 1))
        nc.tensor.matmul(o1[:], Vp1[:, pjp, :, 0:65], pq1[:, :, :],
                         perf_mode=DR, start=(pjp == 0), stop=(pjp == JP - 1))
        st = finalize_a(pic, o0, o1)
        warm = fp.tile([HD, 512], f32, tag="fp", name="warm")
        for h in range(2):
            norm_quantum(pic, st, h)
            # keep the PE activity window alive through the slow norm chain
            for _ in range(4):
                nc.tensor.matmul(warm[:], onesb[64:65, :], KT[64:65, 0:512],
                                 start=True, stop=True, tile_position=(64, 0))
        ypss = []
        for mt in range(4):
            ypss.append(yproj_mm(pic, mt))
            if mt >= 1:
                yproj_evict(pic, mt - 1, ypss[mt - 1],
                            "s" if (mt - 1) % 2 == 0 else "v")
        yproj_evict(pic, 3, ypss[3], "v")

    nc.compile()
    _BUILD_CACHE["k"] = nc
    return nc


def _prep_inputs(x, qkv_w, qkv_b, out_w):
    """Per-core input maps. Core c: batch c//4, head-pair c%4.
    Head1 (odd head) of each pair gets its Q rows scaled by ALPHA for the
    DVE cubic-exp path."""
    dt = np.dtype(ml_dtypes.bfloat16)

    x = np.asarray(x, np.float32)
    qkv_w = np.asarray(qkv_w, np.float32)
    qkv_b = np.asarray(qkv_b, np.float32)
    out_w = np.asarray(out_w, np.float32)

    xts = []
    for b in range(B):
        xt = np.ascontiguousarray(x[b].T).reshape(KT_TILES, P, N)
        xts.append(xt.astype(dt))

    qscale = np.concatenate([np.full(HD, 0.125, np.float32),
                             np.full(HD, 0.125 * ALPHA, np.float32)])
    in_maps = []
    for c in range(NCORES):
        b, m = divmod(c, 4)
        rs = slice(P * m, P * (m + 1))
        wq = (qscale[:, None] * qkv_w[0:D][rs]).T.reshape(KT_TILES, P, P)
        wk = qkv_w[D:2 * D][rs].T.reshape(KT_TILES, P, P)
        wv = qkv_w[2 * D:3 * D][rs].T.reshape(KT_TILES, P, P)
        wo = np.ascontiguousarray(out_w[:, rs].T).reshape(2, HD, D)
        in_maps.append({
            "xt": xts[b],
            "wqt": np.ascontiguousarray(wq).astype(dt),
            "wkt": np.ascontiguousarray(wk).astype(dt),
            "wvt": np.ascontiguousarray(wv).astype(dt),
            "wot": wo.astype(dt),
            "bq": (qscale * qkv_b[0:D][rs]).reshape(P, 1).astype(np.float32),
            "bk": qkv_b[D:2 * D][rs].reshape(P, 1).astype(np.float32),
        })
    return in_maps


def _gather(results, qkv_b, out_w, out_b):
    # y[b] = (sum over the batch's 4 cores of yp)^T + out_w @ bv + out_b
    bias_vec = out_w.astype(np.float32) @ np.asarray(qkv_b, np.float32)[2 * D:3 * D] \
        + np.asarray(out_b, np.float32)
    y = np.empty((B, N, D), np.float32)
    for b in range(B):
        acc = np.zeros((D, N), np.float32)
        for m in range(4):
            acc += np.asarray(results[4 * b + m]["yp"], np.float32).reshape(D, N)
        y[b] = acc.T + bias_vec
    return y


def _run(inputs, trace=False, tmpdir=None):
    from concourse.bass_utils import run_bass_kernel_spmd

    nc = _build()
    in_maps = _prep_inputs(inputs["x"], inputs["qkv_w"], inputs["qkv_b"],
                           inputs["out_w"])
    kw = {}
    if trace:
        kw = dict(trace=True, tmpdir=tmpdir)
    res = run_bass_kernel_spmd(nc, in_maps, core_ids=list(range(NCORES)), **kw)
    y = _gather(res.results, inputs["qkv_b"], inputs["out_w"], inputs["out_b"])
    return y, res


def kernel(x, qkv_w, qkv_b, out_w, out_b):
    y, _ = _run(dict(x=x, qkv_w=qkv_w, qkv_b=qkv_b, out_w=out_w, out_b=out_b))
    return y


# revision 13
# speedup vs baseline: 1.0393x; 1.0393x over previous
"""Multi-head self-attention Trainium2 kernel (8 NeuronCores, SPMD).

Problem: B=2, N=4096, D=512, H=8 heads of dim 64.
  qkv = x @ qkv_w.T + qkv_b ; per-head attention with softmax(QK^T/8) ;
  out = attn @ out_w.T + out_b

Sharding: 16 (batch, head) pairs -> 8 cores, each core owns one batch b and
one head-PAIR (2 adjacent heads = a 128-row slice of the qkv projections).
Each core computes the full attention for its 2 heads over all 4096 rows and
a partial output projection; the host sums the 4 per-batch partials and adds
the (folded) biases.

Pipeline design (v2 — dual-engine softmax + fp8 PV):
  The kernel is exp-throughput-bound: 33.5M exps/core. ScalarE ACT does
  head0's scores, the Vector engine does head1's via a custom 7-stage DVE op
  computing ((y+u)^2+v)^4*(y+a)^4 = K*exp(S) (monic cubic in y = alpha*S,
  alpha folded into head1's Q projection; the per-head softmax normalization
  cancels K). Both engines write p in fp8-e4m3; V is also e4m3, packed
  [128,16,2,80] so the PV matmul runs DoubleRow (2 key-tiles, 256 virtual
  contraction rows, 0.5 cy/row) — PE cost of attention halves.
  Scores S stay bf16xbf16->fp32; the two heads' S matmuls are row-packed at
  tile_position 0/64 and run concurrently. Softmax denominators ride a fused
  ones-column in V; normalization uses a bf16 PE outer-product broadcast,
  DVE reciprocal_approx_fast, and a gpsimd multiply (the only engine with
  slack). Finalize work (PSUM evictions, y-projection evictions, bias adds)
  is placed on ScalarE, which has more slack than DVE in steady state.
"""

import os
import numpy as np
import ml_dtypes

B, N, D, H, HD = 2, 4096, 512, 8, 64
NCORES = 8
KT_TILES = 4      # D / 128 contraction tiles
JT = 32           # N / 128 key tiles
JP = 16           # N / 256 key-tile pairs
ICH = 8           # N / 512 query chunks
P = 128

MODE = "v2-fp8pv"

# exp cubic constants: q(y)=(y+CA)((y+CU)^2+CV); q^4 = K*exp(y/ALPHA),
# max rel err 6.5e-3 over |S|<=3.  K=3.1575 cancels in softmax.
ALPHA = 0.15
CA = 1.02637385
CU = 0.45424912
CV = 1.09069782

_BUILD_CACHE = {}


def _register_exp_op():
    import concourse.dve_ops as dve_ops
    for op in dve_ops.OPS:
        if op.name == "EXP4_POLY_ANT":
            return op
    from concourse.dve_spec import Spec, Src0, C0, C1, C2, sq, lower, _has_src1
    from concourse.dve_uop import DveOpSpec

    t = Src0 + C0
    body = sq(sq((sq(t) + C1) * (Src0 + C2)))

    def ref(in0, in1, s0, s1, imm2):
        x = in0.astype(np.float32)
        return ((((x + s0) ** 2 + s1) * (x + imm2)) ** 4).astype(np.float32)

    spec = Spec(body=body, reference=ref)
    name = "EXP4_POLY_ANT"
    row = max(dve_ops._SUB_OPCODE_FOR_NAME.values()) + 1
    assert row < 0x20
    dve_ops._SUB_OPCODE_FOR_NAME[name] = row
    shas = {}
    for ver in ("v3", "v4"):
        try:
            uops = lower(spec, ver=ver)
            tmp = DveOpSpec(name=name, opcode=row, uops=uops,
                            rd1_en=_has_src1(spec))
            shas[ver] = tmp.sha(ver)
        except Exception:
            pass
    op = dve_ops.DveOp(name, spec, subdim=False, uops_sha=shas)
    dve_ops.OPS.append(op)
    dve_ops.CUSTOM_DVE_SPECS[name] = spec
    return op


def _build():
    """Build (and cache) the compiled Bass program for all cores (SPMD)."""
    if "k" in _BUILD_CACHE:
        return _BUILD_CACHE["k"]

    import concourse.bacc as bacc
    import concourse.mybir as mybir
    import concourse.tile as tile
    from concourse.bass import _add_dep_helper
    from contextlib import ExitStack

    f32 = mybir.dt.float32
    bf16 = mybir.dt.bfloat16
    e4 = mybir.dt.float8e4
    Exp = mybir.ActivationFunctionType.Exp
    Copy = mybir.ActivationFunctionType.Copy
    Ident = mybir.ActivationFunctionType.Identity
    DR = mybir.MatmulPerfMode.DoubleRow
    EXP4 = _register_exp_op()

    nc = bacc.Bacc(None, target_bir_lowering=False)
    xt_d = nc.dram_tensor("xt", [KT_TILES, P, N], bf16, kind="ExternalInput")
    wqt_d = nc.dram_tensor("wqt", [KT_TILES, P, P], bf16, kind="ExternalInput")
    wkt_d = nc.dram_tensor("wkt", [KT_TILES, P, P], bf16, kind="ExternalInput")
    wvt_d = nc.dram_tensor("wvt", [KT_TILES, P, P], bf16, kind="ExternalInput")
    wot_d = nc.dram_tensor("wot", [2, HD, D], bf16, kind="ExternalInput")
    bq_d = nc.dram_tensor("bq", [P, 1], f32, kind="ExternalInput")
    bk_d = nc.dram_tensor("bk", [P, 1], f32, kind="ExternalInput")
    yp_d = nc.dram_tensor("yp", [KT_TILES, P, N], bf16, kind="ExternalOutput")

    def ics(i):
        return slice(i * 512, (i + 1) * 512)

    def jts(j):
        return slice(j * P, (j + 1) * P)

    def mts(m):
        return slice(m * P, (m + 1) * P)

    with tile.TileContext(nc) as tc, ExitStack() as ctx:
        const = ctx.enter_context(tc.tile_pool(name="const", bufs=1))
        # PSUM budget (8 banks): sp 2x[128,1024]f32 = 4, o-pool 2x[65,512] = 2,
        # fp scratch 2x(<=1 bank) = 2.
        sp0 = ctx.enter_context(tc.tile_pool(name="sp0", bufs=2, space="PSUM"))
        sp1 = ctx.enter_context(tc.tile_pool(name="sp1", bufs=2, space="PSUM"))
        op = ctx.enter_context(tc.tile_pool(name="opool", bufs=2, space="PSUM"))
        fp = ctx.enter_context(tc.tile_pool(name="fpool", bufs=2, space="PSUM"))
        pp0 = ctx.enter_context(tc.tile_pool(name="pp0", bufs=4))
        pp1 = ctx.enter_context(tc.tile_pool(name="pp1", bufs=4))
        yep0 = ctx.enter_context(tc.tile_pool(name="yep0", bufs=2))
        yep1 = ctx.enter_context(tc.tile_pool(name="yep1", bufs=2))
        osp0 = ctx.enter_context(tc.tile_pool(name="osp0", bufs=2))
        osp1 = ctx.enter_context(tc.tile_pool(name="osp1", bufs=2))
        rbp = ctx.enter_context(tc.tile_pool(name="rbpool", bufs=2))

        xt = const.tile([P, KT_TILES, N], bf16, tag="xt")
        wqt = const.tile([P, KT_TILES, P], bf16, tag="wqt")
        wkt = const.tile([P, KT_TILES, P], bf16, tag="wkt")
        wvt = const.tile([P, KT_TILES, P], bf16, tag="wvt")
        for k in range(KT_TILES):
            nc.gpsimd.dma_start(wqt[:, k, :], wqt_d[k])
            nc.gpsimd.dma_start(wkt[:, k, :], wkt_d[k])
            nc.scalar.dma_start(wvt[:, k, :], wvt_d[k])
        # x^T in column-major chunk order on ONE queue: the first column
        # blocks (all k-tiles) land at ~12% of the transfer so Q/K projections
        # and the attention loop start early.
        XCH = 1024
        for c in range(N // XCH):
            for k in range(KT_TILES):
                nc.sync.dma_start(xt[:, k, c * XCH:(c + 1) * XCH],
                                  xt_d[k][:, c * XCH:(c + 1) * XCH])
        wot = const.tile([HD, 2, D], bf16, tag="wot")
        for h in range(2):
            nc.scalar.dma_start(wot[:, h, :], wot_d[h])
        bq = const.tile([P, 1], f32, tag="bq")
        bk = const.tile([P, 1], f32, tag="bk")
        nc.gpsimd.dma_start(bq[:], bq_d[:])
        nc.gpsimd.dma_start(bk[:], bk_d[:])

        QT = const.tile([P, N], bf16, tag="QT")
        KT = const.tile([P, N], bf16, tag="KT")
        # V in fp8, DoubleRow pair layout: [p, key-pair, ko, 64 v + 1 ones + pad]
        Vp0 = const.tile([P, JP, 2, 80], e4, tag="Vp0")
        Vp1 = const.tile([P, JP, 2, 80], e4, tag="Vp1")
        OT0 = const.tile([HD, N], bf16, tag="OT0")
        OT1 = const.tile([HD, N], bf16, tag="OT1")
        onesb = const.tile([65, HD], bf16, tag="onesb")
        nc.vector.memset(onesb[64:65, :], 1.0)
        actwarm = const.tile([1, 1], f32, tag="actwarm")
        nc.vector.memset(actwarm[:], 0.0)
        nc.scalar.activation(actwarm[:], actwarm[:], Exp)
        for _ko in range(2):
            nc.vector.memset(Vp0[:, :, _ko, 64:65], 1.0)
            nc.vector.memset(Vp1[:, :, _ko, 64:65], 1.0)

        # ---- projection units ----
        def qproj_unit(ic):
            ps = fp.tile([P, 512], f32, tag="fp", name=f"qp_{ic}")
            for k in range(KT_TILES):
                nc.tensor.matmul(ps[:], wqt[:, k, :], xt[:, k, ics(ic)],
                                 start=(k == 0), stop=(k == KT_TILES - 1))
            nc.vector.tensor_scalar_add(QT[:, ics(ic)], ps[:], bq[:, 0:1])

        def qproj_quarter(ic, q, after=None):
            # one [128,128] N-slice of the Q^T projection; bias-add on ScalarE
            qs = slice(ics(ic).start + 128 * q, ics(ic).start + 128 * (q + 1))
            ps = fp.tile([P, 128], f32, tag="fp", name=f"qq_{ic}_{q}")
            for k in range(KT_TILES):
                mm = nc.tensor.matmul(ps[:], wqt[:, k, :], xt[:, k, qs],
                                      start=(k == 0), stop=(k == KT_TILES - 1))
                if after is not None and k == 0:
                    _add_dep_helper(mm.ins, after.ins, sync=False,
                                    reason="defer qproj behind attention")
            if q < 2:
                nc.scalar.activation(QT[:, qs], ps[:], Ident, bias=bq[:, 0:1])
            else:
                nc.vector.tensor_scalar_add(QT[:, qs], ps[:], bq[:, 0:1])

        def kproj_unit(jc, eng="v"):
            ps = fp.tile([P, 512], f32, tag="fp", name=f"kp_{jc}")
            for k in range(KT_TILES):
                nc.tensor.matmul(ps[:], wkt[:, k, :], xt[:, k, ics(jc)],
                                 start=(k == 0), stop=(k == KT_TILES - 1))
            if eng == "v":
                nc.vector.tensor_scalar_add(KT[:, ics(jc)], ps[:], bk[:, 0:1])
            else:
                nc.scalar.activation(KT[:, ics(jc)], ps[:], Ident, bias=bk[:, 0:1])

        def vproj_unit(jt):
            # V[jt] natural layout -> fp8 pair-packed slabs (one evict per
            # engine so neither stalls the exp cadence)
            jp_, ko = divmod(jt, 2)
            ps = fp.tile([P, P], f32, tag="fp", name=f"vp_{jt}")
            for k in range(KT_TILES):
                nc.tensor.matmul(ps[:], xt[:, k, jts(jt)], wvt[:, k, :],
                                 start=(k == 0), stop=(k == KT_TILES - 1))
            nc.scalar.activation(Vp0[:, jp_, ko, 0:64], ps[:, 0:64], Copy)
            nc.vector.tensor_copy(Vp1[:, jp_, ko, 0:64], ps[:, 64:128])

        # ---- finalize quanta ----
        def finalize_a(ic, o0, o1):
            # evict unnormalized O^T (+denominator row 64) to bf16 SBUF.
            # ScalarE for head0, DVE for head1 (one each, keeps cadence).
            os0 = osp0.tile([65, 512], bf16, tag="os0", name=f"os0_{ic}")
            os1 = osp1.tile([65, 512], bf16, tag="os1", name=f"os1_{ic}")
            nc.scalar.activation(os0[:], o0[:], Copy)
            nc.vector.tensor_copy(os1[:], o1[:])
            return (os0, os1)

        def norm_quantum(ic, st, h):
            # one head: broadcast denominator via bf16 PE outer product,
            # approx-reciprocal on DVE, normalize-multiply on gpsimd.
            os_ = st[h]
            OTt = (OT0, OT1)[h]
            rb = fp.tile([HD, 512], f32, tag="fp", name=f"rb_{ic}_{h}")
            nc.tensor.matmul(rb[:], onesb[64:65, :], os_[64:65, :],
                             start=True, stop=True, tile_position=(64, 0))
            rbs = rbp.tile([HD, 512], f32, tag="rbs", name=f"rbs_{ic}_{h}")
            nc.vector.reciprocal_approx_fast(out=rbs[:], in_=rb[:])
            nc.gpsimd.tensor_mul(OTt[:, ics(ic)], os_[0:64, :], rbs[:])

        def yproj_mm(ic, mt, after=None):
            # one [128,512] slice of the partial output projection (PE only)
            hs = ics(ic)
            yps = fp.tile([P, 512], f32, tag="fp", name=f"yp_{ic}_{mt}")
            mm = nc.tensor.matmul(yps[:], wot[:, 0, mts(mt)], OT0[:, hs],
                                  start=True, stop=False)
            if after is not None:
                _add_dep_helper(mm.ins, after.ins, sync=False,
                                reason="defer finalize yproj behind attention")
            nc.tensor.matmul(yps[:], wot[:, 1, mts(mt)], OT1[:, hs],
                             start=False, stop=True)
            return yps

        def yproj_evict(ic, mt, yps, eng):
            if eng == "s":
                ye = yep0.tile([P, 512], bf16, tag="ye0", name=f"ye_{ic}_{mt}")
                nc.scalar.activation(ye[:], yps[:], Copy)
            else:
                ye = yep1.tile([P, 512], bf16, tag="ye1", name=f"ye_{ic}_{mt}")
                nc.vector.tensor_copy(ye[:], yps[:])
            nc.sync.dma_start(yp_d[mt, :, ics(ic)], ye[:])

        # prologue: K chunk 0 first (needed by the first S tiles), then Q;
        # K chunk 1 is deferred into the loop (needed from jt=4)
        kproj_unit(0)
        qproj_unit(0)

        # finalize schedule within the NEXT chunk (jt slots)
        NORM_JT = {2: 0, 3: 1}
        YP_MM_JT = {10: 0, 12: 1, 14: 2, 16: 3}
        YP_EV_JT = {12: 0, 14: 1, 16: 2, 18: 3}

        otiles = {}
        pend_pv = None       # (p3, jp, ic) pair whose PV is not yet emitted
        pend_b = None
        for ic in range(ICH):
            otiles[ic] = (op.tile([65, 512], f32, tag="o", name=f"o0_{ic}"),
                          op.tile([65, 512], f32, tag="o", name=f"o1_{ic}"))
            cur_p3 = None
            for jt in range(JT):
                jp_, ko = divmod(jt, 2)
                s0 = sp0.tile([P, 512], f32, tag="s0")
                s1 = sp1.tile([P, 512], f32, tag="s1")
                last_s = nc.tensor.matmul(s0[:], KT[0:64, jts(jt)],
                                          QT[0:64, ics(ic)],
                                          start=True, stop=True,
                                          tile_position=(0, 0))
                nc.tensor.matmul(s1[:], KT[64:128, jts(jt)],
                                 QT[64:128, ics(ic)],
                                 start=True, stop=True, tile_position=(64, 0))
                if ko == 0:
                    cp0 = pp0.tile([P, 2, 512], e4, tag="p0")
                    cp1 = pp1.tile([P, 2, 512], e4, tag="p1")
                nc.scalar.activation(cp0[:, ko, :], s0[:], Exp)
                nc.vector._custom_dve(EXP4, out=cp1[:, ko, :], in0=s1[:],
                                      s0=CU, s1=CV, imm2=CA)
                if ko == 1:
                    if pend_pv is not None:
                        pq0, pq1, pjp, pic = pend_pv
                        o0, o1 = otiles[pic]
                        nc.tensor.matmul(o0[:], Vp0[:, pjp, :, 0:65],
                                         pq0[:, :, :], perf_mode=DR,
                                         start=(pjp == 0), stop=(pjp == JP - 1))
                        nc.tensor.matmul(o1[:], Vp1[:, pjp, :, 0:65],
                                         pq1[:, :, :], perf_mode=DR,
                                         start=(pjp == 0), stop=(pjp == JP - 1))
                        if pjp == JP - 1:
                            pend_b = (pic, finalize_a(pic, o0, o1), [None] * 4)
                    pend_pv = (cp0, cp1, jp_, ic)
                # deferred projection work, spread across the loop
                if ic == 0:
                    if jt == 0:
                        kproj_unit(1, eng="v")
                        vproj_unit(0)
                    elif jt == 1:
                        vproj_unit(1)
                        vproj_unit(2)
                    elif jt <= JT - 2:
                        vproj_unit(jt + 1)
                    if 2 <= jt < 26 and jt % 4 == 2:
                        kproj_unit(2 + (jt - 2) // 4,
                                   eng="s" if (jt - 2) % 8 else "v")
                if pend_b is not None:
                    bic, st, ypss = pend_b
                    h = NORM_JT.get(jt)
                    if h is not None:
                        norm_quantum(bic, st, h)
                    mt = YP_MM_JT.get(jt)
                    if mt is not None:
                        ypss[mt] = yproj_mm(bic, mt, after=last_s)
                    mt = YP_EV_JT.get(jt)
                    if mt is not None:
                        yproj_evict(bic, mt, ypss[mt], "s" if mt % 2 == 0 else "v")
                        if mt == 3:
                            pend_b = None
                if 20 <= jt < 24 and ic + 1 < ICH:
                    qproj_quarter(ic + 1, jt - 20, after=last_s)
        # drain the pipeline tail
        pq0, pq1, pjp, pic = pend_pv
        o0, o1 = otiles[pic]
        nc.tensor.matmul(o0[:], Vp0[:, pjp, :, 0:65], pq0[:, :, :],
                         perf_mode=DR, start=(pjp == 0), stop=(pjp == JP - 1))
        nc.tensor.matmul(o1[:], Vp1[:, pjp, :, 0:65], pq1[:, :, :],
                         perf_mode=DR, start=(pjp == 0), stop=(pjp == JP - 1))
        st = finalize_a(pic, o0, o1)
        warm = fp.tile([HD, 512], f32, tag="fp", name="warm")
        for h in range(2):
            norm_quantum(pic, st, h)
            # keep the PE activity window alive through the slow norm chain
            for _ in range(4):
                nc.tensor.matmul(warm[:], onesb[64:65, :], KT[64:65, 0:512],
                                 start=True, stop=True, tile_position=(64, 0))
        ypss = []
        for mt in range(4):
            ypss.append(yproj_mm(pic, mt))
            if mt >= 1:
                yproj_evict(pic, mt - 1, ypss[mt - 1],
                            "s" if (mt - 1) % 2 == 0 else "v")
        yproj_evict(pic, 3, ypss[3], "v")

    nc.compile()
    _BUILD_CACHE["k"] = nc
    return nc


def _prep_inputs(x, qkv_w, qkv_b, out_w):
    """Per-core input maps. Core c: batch c//4, head-pair c%4.
    Head1 (odd head) of each pair gets its Q rows scaled by ALPHA for the
    DVE cubic-exp path."""
    dt = np.dtype(ml_dtypes.bfloat16)

    x = np.asarray(x, np.float32)
    qkv_w = np.asarray(qkv_w, np.float32)
    qkv_b = np.asarray(qkv_b, np.float32)
    out_w = np.asarray(out_w, np.float32)

    xts = []
    for b in range(B):
        xt = np.ascontiguousarray(x[b].T).reshape(KT_TILES, P, N)
        xts.append(xt.astype(dt))

    qscale = np.concatenate([np.full(HD, 0.125, np.float32),
                             np.full(HD, 0.125 * ALPHA, np.float32)])
    in_maps = []
    for c in range(NCORES):
        b, m = divmod(c, 4)
        rs = slice(P * m, P * (m + 1))
        wq = (qscale[:, None] * qkv_w[0:D][rs]).T.reshape(KT_TILES, P, P)
        wk = qkv_w[D:2 * D][rs].T.reshape(KT_TILES, P, P)
        wv = qkv_w[2 * D:3 * D][rs].T.reshape(KT_TILES, P, P)
        wo = np.ascontiguousarray(out_w[:, rs].T).reshape(2, HD, D)
        in_maps.append({
            "xt": xts[b],
            "wqt": np.ascontiguousarray(wq).astype(dt),
            "wkt": np.ascontiguousarray(wk).astype(dt),
            "wvt": np.ascontiguousarray(wv).astype(dt),
            "wot": wo.astype(dt),
            "bq": (qscale * qkv_b[0:D][rs]).reshape(P, 1).astype(np.float32),
            "bk": qkv_b[D:2 * D][rs].reshape(P, 1).astype(np.float32),
        })
    return in_maps


def _gather(results, qkv_b, out_w, out_b):
    # y[b] = (sum over the batch's 4 cores of yp)^T + out_w @ bv + out_b
    bias_vec = out_w.astype(np.float32) @ np.asarray(qkv_b, np.float32)[2 * D:3 * D] \
        + np.asarray(out_b, np.float32)
    y = np.empty((B, N, D), np.float32)
    for b in range(B):
        acc = np.zeros((D, N), np.float32)
        for m in range(4):
            acc += np.asarray(results[4 * b + m]["yp"], np.float32).reshape(D, N)
        y[b] = acc.T + bias_vec
    return y


def _run(inputs, trace=False, tmpdir=None):
    from concourse.bass_utils import run_bass_kernel_spmd

    nc = _build()
    in_maps = _prep_inputs(inputs["x"], inputs["qkv_w"], inputs["qkv_b"],
                           inputs["out_w"])
    kw = {}
    if trace:
        kw = dict(trace=True, tmpdir=tmpdir)
    res = run_bass_kernel_spmd(nc, in_maps, core_ids=list(range(NCORES)), **kw)
    y = _gather(res.results, inputs["qkv_b"], inputs["out_w"], inputs["out_b"])
    return y, res


def kernel(x, qkv_w, qkv_b, out_w, out_b):
    y, _ = _run(dict(x=x, qkv_w=qkv_w, qkv_b=qkv_b, out_w=out_w, out_b=out_b))
    return y
